# revision 28
# baseline (speedup 1.0000x reference)
"""Trainium2 Bass kernel for nn_Block2x2DiagProduct (butterfly product).

Strategy:
  Stages 1..9 of the butterfly (all with block size <= 512) compose into
  blockdiag(R, R) with a single dense 512x512 matrix R shared by both
  halves (parameters are shared across blocks within each factor). The
  final stage (block size 1024) is a columnwise 2x2 butterfly:

      out[:, k]     = A[k]*y[:, k] + B[k]*y[:, 512+k]
      out[:, 512+k] = C[k]*y[:, k] + D[k]*y[:, 512+k]

  where y = x @ blockdiag(R^T, R^T). So the device kernel is two K=512
  matmuls per row tile (PE) plus six columnwise multiply/adds (split
  across Vector and GpSimd, with Scalar doing the PSUM->SBUF staging).
  This halves the PE matmul work vs composing one dense 1024x1024
  matrix, moving the peeled stage to otherwise-idle engines.

  Dtype choices (walrus forbids mixing 32-bit and 16-bit matmul
  operands, so each matmul is uniform):
  - x and the identity are float32r (same bits as f32): the PE
    transposes run at 1.5 cycles/row instead of 2.0 for plain f32.
  - The transposed x is staged PSUM->SBUF as bf16 and W is shipped
    bf16 from the host: all-bf16 matmuls run at 1 cycle/row (same rate
    as f32r), W's DMA halves, and the four startup cast copies of W
    disappear.
  - The stage-0 coefficients ship bf16 (half the DMA); the butterfly
    multiplies read y straight from PSUM in f32.

  R is composed on the host in float64 (9 einsums over a 512x512
  identity). Sharding: pure data parallel — batch dim of x split across
  8 cores; W (0.5 MiB) and the stage-0 coefficients are replicated.

  Per-core per 128-row tile of x:
    - HWDGE DMA in; PE-transposes the 8 [128,128] feature chunks 4-up
      into [128,512] PSUM tiles (matmul contracts along partitions, so
      x needs features on partitions); Scalar-engine cast copies move
      them to SBUF as bf16.
    - 8 accumulating bf16 matmuls -> y_lo, y_hi in PSUM.
    - Butterfly: Vector computes A*y_lo + B*y_hi (reading PSUM), GpSimd
      computes C*y_lo + D*y_hi from the Vector-made products, both into
      the output tile; HWDGE DMA out.
"""

import os
import sys

for _p in ("/opt/trn_rl_repo", "/root/.axon_site/_ro/trn_rl_repo"):
    if os.path.isdir(_p) and _p not in sys.path:
        sys.path.insert(0, _p)

import numpy as np
import ml_dtypes

import concourse.bacc as bacc
import concourse.bass as bass
import concourse.mybir as mybir
from concourse.bass_utils import run_bass_kernel_spmd
from concourse.masks import make_identity
from concourse.tile import TileContext

SIZE = 1024
HALF = SIZE // 2
M = 10  # number of butterfly factors
N_CORES = 8
P = 128
KC = HALF // P  # 4 contraction chunks per half

# Results of the last device run (for the test harness).
last_exec_time_ns = None
last_mean_exec_time_ns = None

_nc_cache = {}


def _compose_w1t(params):
    """Compose butterfly stages 1..9 into W1t (512x512, f64) such that
    y_half = x_half @ W1t for each 512 half. Both halves share W1t because
    each factor's parameters are shared across its blocks."""
    w = np.eye(HALF, dtype=np.float64)
    for i in reversed(range(1, M)):
        s = SIZE >> i
        y = w.reshape(HALF, HALF // s, 2, s // 2)
        w = np.einsum(
            "ijk,bnjk->bnik", params[i].astype(np.float64), y
        ).reshape(HALF, HALF)
    return w


def _build_nc(rows):
    f32 = mybir.dt.float32
    f32r = mybir.dt.float32r
    bf16 = mybir.dt.bfloat16
    nb = rows // P

    # Bacc (not raw Bass): its finalize() pipeline splits multi-sem waits
    # into EventSemaphore instructions (HW allows 1 sync-wait per inst).
    nc = bacc.Bacc(None, target_bir_lowering=False)
    x_d = nc.dram_tensor("x", [rows, SIZE], f32r, kind="ExternalInput")
    w_d = nc.dram_tensor("w", [HALF, HALF], bf16, kind="ExternalInput")
    coef_d = nc.dram_tensor("coef", [P, 4, HALF], bf16, kind="ExternalInput")
    o_d = nc.dram_tensor("o", [rows, SIZE], f32, kind="ExternalOutput")

    with TileContext(nc) as tc:
        with (
            tc.tile_pool(name="const", bufs=1) as const_pool,
            tc.tile_pool(name="xin", bufs=4) as xpool,
            tc.tile_pool(name="xt", bufs=8) as xtpool,
            tc.tile_pool(name="stage", bufs=6) as spool,
            tc.tile_pool(name="osb", bufs=8) as opool,
            tc.tile_pool(name="tpsum", bufs=4, space="PSUM") as tpsum,
            tc.tile_pool(name="mpsum", bufs=4, space="PSUM") as mpsum,
        ):
            ident_f = const_pool.tile([P, P], f32)
            make_identity(nc, ident_f[:])
            # GpSimd memset can't write float32r tiles, so build the
            # identity in f32 and cast-copy once.
            ident = const_pool.tile([P, P], f32r)
            nc.vector.tensor_copy(out=ident[:], in_=ident_f[:])
            # Dummy PE op consuming the identity: walrus allows only one
            # sync-wait on (transpose-)matmuls, and without this the first
            # real transpose would need two (identity-ready + x-DMA).
            pst0 = tpsum.tile([P, P], f32r, name="pst_warm", tag="pst")
            nc.tensor.transpose(pst0[:], ident[:], ident[:])

            # W1t resident in SBUF (bf16, direct DMA, no cast): partition
            # p, chunk c holds W1t[c*128+p, :]. Loads on the GpSimd HWDGE
            # queue, which is otherwise idle, so neither the x loads (SP
            # queue) nor the stores (ACT queue) are delayed at startup.
            w_sb = const_pool.tile([P, KC, HALF], bf16)
            nc.gpsimd.dma_start(
                out=w_sb[:],
                in_=w_d[:, :].rearrange("(c p) f -> p c f", p=P),
            )
            # Stage-0 coefficients A,B,C,D, pre-replicated across
            # partitions (bf16).
            coef_sb = const_pool.tile([P, 4, HALF], bf16)
            nc.gpsimd.dma_start(out=coef_sb[:], in_=coef_d[:, :, :])

            for bp in range(nb // 2):
                # Two 128-row tiles per DMA: 1 MiB transfers are the DMA
                # bandwidth sweet spot and halve the DMA op count. bufs=4
                # keeps the slot-WAW predecessor on the own HWDGE lane so
                # the load fits the DMA struct's sync-wait limit.
                x_sb = xpool.tile([P, 2, SIZE], f32r)
                nc.sync.dma_start(
                    out=x_sb[:],
                    in_=x_d[bp * 2 * P : (bp + 1) * 2 * P, :].rearrange(
                        "(j p) f -> p j f", p=P
                    ),
                )
                o_sbs = []
                for j in range(2):
                    # Transpose 8 chunks of [128b, 128f] -> [128f, 128b],
                    # 4 chunks per PSUM bank, one Scalar-engine cast each.
                    xts = []
                    for h in range(2):
                        pst = tpsum.tile(
                            [P, HALF], f32r, tag="pst", name=f"pst{h}"
                        )
                        for c in range(KC):
                            k = KC * h + c
                            nc.tensor.transpose(
                                pst[:, c * P : (c + 1) * P],
                                x_sb[:, j, k * P : (k + 1) * P],
                                ident[:],
                            )
                        xt_h = xtpool.tile(
                            [P, HALF], bf16, tag="xt", name=f"xt{h}"
                        )
                        nc.scalar.copy(out=xt_h[:], in_=pst[:])
                        xts.append(xt_h)
                    # y_half[b, :] = sum_k x_half[b, k] * W1t[k, :]
                    psos = [
                        mpsum.tile([P, HALF], f32, tag="mm_psum", name=f"pso{h}")
                        for h in range(2)
                    ]
                    for c in range(KC):
                        for h in range(2):
                            nc.tensor.matmul(
                                psos[h][:],
                                xts[h][:, c * P : (c + 1) * P],
                                w_sb[:, c, :],
                                start=(c == 0),
                                stop=(c == KC - 1),
                            )
                    # Peeled stage 0: out_lo = A*y_lo + B*y_hi, out_hi =
                    # C*y_lo + D*y_hi. Vector does all four multiplies
                    # straight from PSUM (GpSimd cannot read PSUM); GpSimd
                    # does the two adds from SBUF.
                    t0 = spool.tile([P, HALF], f32, tag="t0", name="t0")
                    t1 = spool.tile([P, HALF], f32, tag="t1", name="t1")
                    t2 = spool.tile([P, HALF], f32, tag="t2", name="t2")
                    t3 = spool.tile([P, HALF], f32, tag="t3", name="t3")
                    nc.vector.tensor_mul(t0[:], psos[0][:], coef_sb[:, 0, :])
                    nc.vector.tensor_mul(t1[:], psos[1][:], coef_sb[:, 1, :])
                    nc.vector.tensor_mul(t2[:], psos[0][:], coef_sb[:, 2, :])
                    nc.vector.tensor_mul(t3[:], psos[1][:], coef_sb[:, 3, :])
                    o_sb = opool.tile([P, SIZE], f32)
                    nc.gpsimd.tensor_add(o_sb[:, :HALF], t0[:], t1[:])
                    nc.gpsimd.tensor_add(o_sb[:, HALF:], t2[:], t3[:])
                    o_sbs.append(o_sb)
                # Store per 128-row tile (512 KiB) on the ACT HWDGE queue:
                # loads (SP queue) and stores stream through separate DMA
                # queues, and j=0's store issues as soon as its own adds
                # finish instead of waiting for the pair. Both triggers
                # sit AFTER the j loop so the Scalar engine's in-order
                # stream never stalls an xt copy on a pending store.
                for j in range(2):
                    nc.scalar.dma_start(
                        out=o_d[
                            (bp * 2 + j) * P : (bp * 2 + j + 1) * P, :
                        ],
                        in_=o_sbs[j][:],
                    )
    nc.finalize()
    return nc


def kernel(**inputs):
    global last_exec_time_ns, last_mean_exec_time_ns

    x = np.ascontiguousarray(np.asarray(inputs["x"], dtype=np.float32))
    params = [np.asarray(inputs[f"ABCD{i}"]) for i in range(M)]
    w1t = np.ascontiguousarray(
        _compose_w1t(params).astype(np.float32).astype(ml_dtypes.bfloat16)
    )
    abcd0 = params[0].astype(np.float32)  # (2, 2, 512)
    coef = np.ascontiguousarray(
        np.broadcast_to(
            abcd0.reshape(1, 4, HALF), (P, 4, HALF)
        ).astype(ml_dtypes.bfloat16)
    )

    batch = x.shape[0]
    if batch % (N_CORES * 2 * P) != 0:
        # Shape outside the tiled layout this kernel hardcodes — fall back
        # to a host matmul (correct, just not accelerated).
        full = _compose_w1t(params)
        y_lo = x[:, :HALF].astype(np.float64) @ full
        y_hi = x[:, HALF:].astype(np.float64) @ full
        a, b = params[0][0, 0].astype(np.float64), params[0][0, 1].astype(
            np.float64
        )
        c, dd = params[0][1, 0].astype(np.float64), params[0][1, 1].astype(
            np.float64
        )
        return np.concatenate(
            [a * y_lo + b * y_hi, c * y_lo + dd * y_hi], axis=1
        ).astype(np.float32)
    rows = batch // N_CORES

    if rows not in _nc_cache:
        _nc_cache[rows] = _build_nc(rows)
    nc = _nc_cache[rows]

    in_maps = [
        {"x": x[i * rows : (i + 1) * rows], "w": w1t, "coef": coef}
        for i in range(N_CORES)
    ]
    try:
        res = run_bass_kernel_spmd(nc, in_maps, core_ids=list(range(N_CORES)))
    except Exception:
        # Transient axon/PJRT INTERNAL errors have been observed on the
        # first attempt in a fresh process; one retry clears them.
        res = run_bass_kernel_spmd(nc, in_maps, core_ids=list(range(N_CORES)))
    last_exec_time_ns = res.exec_time_ns
    last_mean_exec_time_ns = res.mean_exec_time_ns

    return np.concatenate([r["o"] for r in res.results], axis=0)


# revision 31
# speedup vs baseline: 1.1080x; 1.1080x over previous
"""Trainium2 Bass kernel for nn_Block2x2DiagProduct (butterfly product).

Strategy:
  Stages 1..9 of the butterfly (all with block size <= 512) compose into
  blockdiag(R, R) with a single dense 512x512 matrix R shared by both
  halves (parameters are shared across blocks within each factor). The
  final stage (block size 1024) is a columnwise 2x2 butterfly:

      out[:, k]     = A[k]*y[:, k] + B[k]*y[:, 512+k]
      out[:, 512+k] = C[k]*y[:, k] + D[k]*y[:, 512+k]

  where y = x @ blockdiag(R^T, R^T). So the device kernel is two K=512
  matmuls per row tile (PE) plus six columnwise multiply/adds (split
  across Vector and GpSimd, with Scalar doing the PSUM->SBUF staging).
  This halves the PE matmul work vs composing one dense 1024x1024
  matrix, moving the peeled stage to otherwise-idle engines.

  Dtype choices (walrus forbids mixing 32-bit and 16-bit matmul
  operands, so each matmul is uniform):
  - x and the identity are float32r (same bits as f32): the PE
    transposes run at 1.5 cycles/row instead of 2.0 for plain f32.
  - The transposed x is staged PSUM->SBUF as bf16 and W is shipped
    bf16 from the host: all-bf16 matmuls run at 1 cycle/row (same rate
    as f32r), W's DMA halves, and the four startup cast copies of W
    disappear.
  - The stage-0 coefficients ship bf16 (half the DMA); the butterfly
    multiplies read y straight from PSUM in f32.

  R is composed on the host in float64 (9 einsums over a 512x512
  identity). Sharding: pure data parallel — batch dim of x split across
  8 cores; W (0.5 MiB) and the stage-0 coefficients are replicated.

  Per-core per 128-row tile of x:
    - HWDGE DMA in; PE-transposes the 8 [128,128] feature chunks 4-up
      into [128,512] PSUM tiles (matmul contracts along partitions, so
      x needs features on partitions); Scalar-engine cast copies move
      them to SBUF as bf16.
    - 8 accumulating bf16 matmuls -> y_lo, y_hi in PSUM.
    - Butterfly: Vector computes A*y_lo + B*y_hi (reading PSUM), GpSimd
      computes C*y_lo + D*y_hi from the Vector-made products, both into
      the output tile; HWDGE DMA out.
"""

import os
import sys

for _p in ("/opt/trn_rl_repo", "/root/.axon_site/_ro/trn_rl_repo"):
    if os.path.isdir(_p) and _p not in sys.path:
        sys.path.insert(0, _p)

import numpy as np
import ml_dtypes

import concourse.bacc as bacc
import concourse.bass as bass
import concourse.mybir as mybir
from concourse.bass_utils import run_bass_kernel_spmd
from concourse.masks import make_identity
from concourse.tile import TileContext

SIZE = 1024
HALF = SIZE // 2
M = 10  # number of butterfly factors
N_CORES = 8
P = 128
KC = HALF // P  # 4 contraction chunks per half

# Results of the last device run (for the test harness).
last_exec_time_ns = None
last_mean_exec_time_ns = None

_nc_cache = {}


def _compose_w1t(params):
    """Compose butterfly stages 1..9 into W1t (512x512, f64) such that
    y_half = x_half @ W1t for each 512 half. Both halves share W1t because
    each factor's parameters are shared across its blocks."""
    w = np.eye(HALF, dtype=np.float64)
    for i in reversed(range(1, M)):
        s = SIZE >> i
        y = w.reshape(HALF, HALF // s, 2, s // 2)
        w = np.einsum(
            "ijk,bnjk->bnik", params[i].astype(np.float64), y
        ).reshape(HALF, HALF)
    return w


def _build_nc(rows):
    f32 = mybir.dt.float32
    f32r = mybir.dt.float32r
    bf16 = mybir.dt.bfloat16
    nb = rows // P

    # Bacc (not raw Bass): its finalize() pipeline splits multi-sem waits
    # into EventSemaphore instructions (HW allows 1 sync-wait per inst).
    nc = bacc.Bacc(None, target_bir_lowering=False)
    x_d = nc.dram_tensor("x", [rows, SIZE], f32r, kind="ExternalInput")
    w_d = nc.dram_tensor("w", [HALF, HALF], bf16, kind="ExternalInput")
    coef_d = nc.dram_tensor("coef", [P, 4, HALF], bf16, kind="ExternalInput")
    o_d = nc.dram_tensor("o", [rows, SIZE], f32, kind="ExternalOutput")

    with TileContext(nc) as tc:
        with (
            tc.tile_pool(name="const", bufs=1) as const_pool,
            tc.tile_pool(name="xin", bufs=4) as xpool,
            tc.tile_pool(name="xt", bufs=8) as xtpool,
            tc.tile_pool(name="stage", bufs=6) as spool,
            tc.tile_pool(name="osb", bufs=6) as opool,
            tc.tile_pool(name="tpsum", bufs=4, space="PSUM") as tpsum,
            tc.tile_pool(name="mpsum", bufs=4, space="PSUM") as mpsum,
        ):
            ident_f = const_pool.tile([P, P], f32)
            make_identity(nc, ident_f[:])
            # GpSimd memset can't write float32r tiles, so build the
            # identity in f32 and cast-copy once.
            ident = const_pool.tile([P, P], f32r)
            nc.vector.tensor_copy(out=ident[:], in_=ident_f[:])
            # Dummy PE op consuming the identity: walrus allows only one
            # sync-wait on (transpose-)matmuls, and without this the first
            # real transpose would need two (identity-ready + x-DMA).
            pst0 = tpsum.tile([P, P], f32r, name="pst_warm", tag="pst")
            nc.tensor.transpose(pst0[:], ident[:], ident[:])

            # W1t resident in SBUF (bf16, direct DMA, no cast): partition
            # p, chunk c holds W1t[c*128+p, :]. Loads on the GpSimd HWDGE
            # queue, which is otherwise idle, so neither the x loads (SP
            # queue) nor the stores (ACT queue) are delayed at startup.
            w_sb = const_pool.tile([P, KC, HALF], bf16)
            nc.gpsimd.dma_start(
                out=w_sb[:],
                in_=w_d[:, :].rearrange("(c p) f -> p c f", p=P),
            )
            # Stage-0 coefficients A,B,C,D, pre-replicated across
            # partitions (bf16).
            coef_sb = const_pool.tile([P, 4, HALF], bf16)
            nc.gpsimd.dma_start(out=coef_sb[:], in_=coef_d[:, :, :])

            for bp in range(nb // 2):
                # Two 128-row tiles per DMA: 1 MiB transfers are the DMA
                # bandwidth sweet spot and halve the DMA op count. bufs=4
                # keeps the slot-WAW predecessor on the own HWDGE lane so
                # the load fits the DMA struct's sync-wait limit.
                x_sb = xpool.tile([P, 2, SIZE], f32r)
                nc.sync.dma_start(
                    out=x_sb[:],
                    in_=x_d[bp * 2 * P : (bp + 1) * 2 * P, :].rearrange(
                        "(j p) f -> p j f", p=P
                    ),
                )
                o_sb = opool.tile([P, 2, SIZE], f32)
                for j in range(2):
                    # Transpose 8 chunks of [128b, 128f] -> [128f, 128b],
                    # 4 chunks per PSUM bank, one Scalar-engine cast each.
                    xts = []
                    for h in range(2):
                        pst = tpsum.tile(
                            [P, HALF], f32r, tag="pst", name=f"pst{h}"
                        )
                        for c in range(KC):
                            k = KC * h + c
                            nc.tensor.transpose(
                                pst[:, c * P : (c + 1) * P],
                                x_sb[:, j, k * P : (k + 1) * P],
                                ident[:],
                            )
                        xt_h = xtpool.tile(
                            [P, HALF], bf16, tag="xt", name=f"xt{h}"
                        )
                        nc.scalar.copy(out=xt_h[:], in_=pst[:])
                        xts.append(xt_h)
                    # y_half[b, :] = sum_k x_half[b, k] * W1t[k, :]
                    psos = [
                        mpsum.tile([P, HALF], f32, tag="mm_psum", name=f"pso{h}")
                        for h in range(2)
                    ]
                    for c in range(KC):
                        for h in range(2):
                            nc.tensor.matmul(
                                psos[h][:],
                                xts[h][:, c * P : (c + 1) * P],
                                w_sb[:, c, :],
                                start=(c == 0),
                                stop=(c == KC - 1),
                            )
                    # Peeled stage 0: out_lo = A*y_lo + B*y_hi, out_hi =
                    # C*y_lo + D*y_hi. Vector does all four multiplies
                    # straight from PSUM (GpSimd cannot read PSUM); GpSimd
                    # does the two adds from SBUF.
                    t0 = spool.tile([P, HALF], f32, tag="t0", name="t0")
                    t1 = spool.tile([P, HALF], f32, tag="t1", name="t1")
                    t2 = spool.tile([P, HALF], f32, tag="t2", name="t2")
                    t3 = spool.tile([P, HALF], f32, tag="t3", name="t3")
                    nc.vector.tensor_mul(t0[:], psos[0][:], coef_sb[:, 0, :])
                    nc.vector.tensor_mul(t1[:], psos[1][:], coef_sb[:, 1, :])
                    nc.vector.tensor_mul(t2[:], psos[0][:], coef_sb[:, 2, :])
                    nc.vector.tensor_mul(t3[:], psos[1][:], coef_sb[:, 3, :])
                    nc.gpsimd.tensor_add(o_sb[:, j, :HALF], t0[:], t1[:])
                    nc.gpsimd.tensor_add(o_sb[:, j, HALF:], t2[:], t3[:])
                # Store on the ACT HWDGE queue so loads (SP queue) and
                # stores stream through separate DMA queues. 1 MiB
                # transfers are the DMA bandwidth sweet spot (per-tile
                # 512 KiB stores measured slower).
                nc.scalar.dma_start(
                    out=o_d[bp * 2 * P : (bp + 1) * 2 * P, :].rearrange(
                        "(j p) f -> p j f", p=P
                    ),
                    in_=o_sb[:],
                )
    nc.finalize()
    return nc


def kernel(**inputs):
    global last_exec_time_ns, last_mean_exec_time_ns

    x = np.ascontiguousarray(np.asarray(inputs["x"], dtype=np.float32))
    params = [np.asarray(inputs[f"ABCD{i}"]) for i in range(M)]
    w1t = np.ascontiguousarray(
        _compose_w1t(params).astype(np.float32).astype(ml_dtypes.bfloat16)
    )
    abcd0 = params[0].astype(np.float32)  # (2, 2, 512)
    coef = np.ascontiguousarray(
        np.broadcast_to(
            abcd0.reshape(1, 4, HALF), (P, 4, HALF)
        ).astype(ml_dtypes.bfloat16)
    )

    batch = x.shape[0]
    if batch % (N_CORES * 2 * P) != 0:
        # Shape outside the tiled layout this kernel hardcodes — fall back
        # to a host matmul (correct, just not accelerated).
        full = _compose_w1t(params)
        y_lo = x[:, :HALF].astype(np.float64) @ full
        y_hi = x[:, HALF:].astype(np.float64) @ full
        a, b = params[0][0, 0].astype(np.float64), params[0][0, 1].astype(
            np.float64
        )
        c, dd = params[0][1, 0].astype(np.float64), params[0][1, 1].astype(
            np.float64
        )
        return np.concatenate(
            [a * y_lo + b * y_hi, c * y_lo + dd * y_hi], axis=1
        ).astype(np.float32)
    rows = batch // N_CORES

    if rows not in _nc_cache:
        _nc_cache[rows] = _build_nc(rows)
    nc = _nc_cache[rows]

    in_maps = [
        {"x": x[i * rows : (i + 1) * rows], "w": w1t, "coef": coef}
        for i in range(N_CORES)
    ]
    try:
        res = run_bass_kernel_spmd(nc, in_maps, core_ids=list(range(N_CORES)))
    except Exception:
        # Transient axon/PJRT INTERNAL errors have been observed on the
        # first attempt in a fresh process; one retry clears them.
        res = run_bass_kernel_spmd(nc, in_maps, core_ids=list(range(N_CORES)))
    last_exec_time_ns = res.exec_time_ns
    last_mean_exec_time_ns = res.mean_exec_time_ns

    return np.concatenate([r["o"] for r in res.results], axis=0)
